# revision 1
# baseline (speedup 1.0000x reference)
"""Trainium2 Bass kernel for nn_CosineLayer (retrieval_knn).

Computes out = concat(normalize(features) @ normalize(weight).T, threshold_col).

Strategy (tensor/vocab parallel on the 434k concept axis, per sharding hint):
  - Host: L2-normalize features and weight rows (cheap / one streaming pass),
    fold normalization into the weight, transpose shards to [K, N_shard] so the
    contraction dim lands on SBUF partitions, pad N to 8*54272.
  - Device (x8 SPMD): pure streaming matmul sim_shard = f_hatT.T @ w_hatT_shard
    in float32r (TF32-like TensorE mode: 1 cycle/row, ~1.6e-4 rel err), PSUM
    accumulation over K=768 (6 chunks of 128), DVE copy PSUM->SBUF, DMA out.
  - Host: concat shard outputs, trim padding, append threshold column.
"""

import numpy as np

import concourse.mybir as mybir
import concourse.tile as tile
from concourse import bacc
from concourse.bass_utils import run_bass_kernel_spmd

N_CORES = 8
B = 256              # feature rows
K = 768              # embedding dim
KC = K // 128        # 6 k-chunks of 128 partitions
N_FULL = 434056      # concept rows
N_SHARD = 54272      # = 106*512; 8*54272 = 434176 (pad 120)
NT = 1024            # n-columns processed per chunk
N_CHUNKS = N_SHARD // NT
EPS = 1e-8

_CACHED = {}


def _build_bass():
    """Build + compile the single-core program (same NEFF runs on all 8 cores)."""
    nc = bacc.Bacc("TRN2", target_bir_lowering=False, debug=False,
                   num_devices=N_CORES)
    mmdt = mybir.dt.float32r
    fT_d = nc.dram_tensor("fT", [K, B], mmdt, kind="ExternalInput").ap()
    wT_d = nc.dram_tensor("wT", [K, N_SHARD], mmdt, kind="ExternalInput").ap()
    out_d = nc.dram_tensor("out", [B, N_SHARD], mybir.dt.float32,
                           kind="ExternalOutput").ap()

    wT_r = wT_d.rearrange("(c p) n -> p c n", p=128)   # [128, KC, N_SHARD]
    fT_r = fT_d.rearrange("(c p) b -> p c b", p=128)   # [128, KC, B]

    with tile.TileContext(nc) as tc:
        with (
            tc.tile_pool(name="fpool", bufs=1) as fpool,
            tc.tile_pool(name="wpool", bufs=3) as wpool,
            tc.tile_pool(name="opool", bufs=3) as opool,
            tc.tile_pool(name="psum", bufs=8, space="PSUM") as psum,
        ):
            fsb = fpool.tile([128, KC, B], mmdt)
            nc.sync.dma_start(fsb[:], fT_r[:])

            for n in range(N_CHUNKS):
                wsb = wpool.tile([128, KC, NT], mmdt)
                nc.sync.dma_start(wsb[:], wT_r[:, :, n * NT:(n + 1) * NT])

                for b in range(B // 128):
                    osb = opool.tile([128, NT], mybir.dt.float32)
                    for h in range(NT // 512):
                        ps = psum.tile([128, 512], mybir.dt.float32)
                        for c in range(KC):
                            nc.tensor.matmul(
                                ps[:],
                                fsb[:, c, b * 128:(b + 1) * 128],
                                wsb[:, c, h * 512:(h + 1) * 512],
                                start=(c == 0),
                                stop=(c == KC - 1),
                            )
                        nc.vector.tensor_copy(osb[:, h * 512:(h + 1) * 512], ps[:])
                    # output DMAs ride the ACT HWDGE ring so they don't
                    # queue behind the next chunk's input DMA on SP
                    nc.scalar.dma_start(
                        out_d[b * 128:(b + 1) * 128, n * NT:(n + 1) * NT], osb[:]
                    )
    nc.compile()
    return nc


def kernel(features, weight, threshold):
    features = np.asarray(features, dtype=np.float32)
    weight = np.asarray(weight, dtype=np.float32)

    f_norm = np.linalg.norm(features, axis=1, keepdims=True)
    f_hat = features / np.maximum(f_norm, EPS)
    fT = np.ascontiguousarray(f_hat.T)                       # [768, 256]

    w_norm = np.linalg.norm(weight, axis=1, keepdims=True)
    w_inv = (1.0 / np.maximum(w_norm, EPS)).astype(np.float32)

    shards = []
    for i in range(N_CORES):
        n0 = i * N_SHARD
        n1 = min(n0 + N_SHARD, N_FULL)
        s = np.empty((K, N_SHARD), dtype=np.float32)
        np.multiply(weight[n0:n1].T, w_inv[n0:n1].T, out=s[:, : n1 - n0])
        if n1 - n0 < N_SHARD:
            s[:, n1 - n0:] = 0.0
        shards.append(s)

    if "nc" not in _CACHED:
        _CACHED["nc"] = _build_bass()
    nc = _CACHED["nc"]

    in_maps = [{"fT": fT, "wT": shards[i]} for i in range(N_CORES)]
    res = run_bass_kernel_spmd(nc, in_maps, core_ids=list(range(N_CORES)))
    _CACHED["last_result"] = res

    out = np.empty((B, N_FULL + 1), dtype=np.float32)
    for i in range(N_CORES):
        n0 = i * N_SHARD
        n1 = min(n0 + N_SHARD, N_FULL)
        out[:, n0:n1] = res.results[i]["out"][:, : n1 - n0]
    out[:, N_FULL] = np.float32(threshold)
    return out


# revision 2
# speedup vs baseline: 1.6665x; 1.6665x over previous
"""Trainium2 Bass kernel for nn_CosineLayer (retrieval_knn).

Computes out = concat(normalize(features) @ normalize(weight).T, threshold_col).

Strategy (tensor/vocab parallel on the 434k concept axis, per sharding hint):
  - Host: L2-normalize features and weight rows (cheap one-pass prep), fold
    normalization into the weight, transpose shards to [K, N_shard] so the
    contraction dim lands on SBUF partitions, pad N to 8*54272.
  - Device (x8 SPMD): pure streaming matmul sim_shard = f_hatT.T @ w_hatT_shard
    (float32r: TF32-like TensorE mode, 1 cycle/row), PSUM accumulation over
    K=768 (6 chunks of 128), DVE copy PSUM->SBUF, DMA out.
  - Host: concat shard outputs, trim padding, append threshold column.
"""

import os

import numpy as np

import concourse.mybir as mybir
import concourse.tile as tile
from concourse import bacc
from concourse.bass_utils import run_bass_kernel_spmd

N_CORES = 8
B = 256              # feature rows
K = 768              # embedding dim
KC = K // 128        # 6 k-chunks of 128 partitions
N_FULL = 434056      # concept rows
N_SHARD = 54272      # = 106*512; 8*54272 = 434176 (pad 120)
NT = 1024            # n-columns processed per chunk
N_CHUNKS = N_SHARD // NT
EPS = 1e-8

# weight/feature compute dtype: fp32r (TF32-like, ~6e-5 err) or fp16/bf16
# (half the HBM traffic, ~6e-4 / ~2.4e-3 err)
MODE = os.environ.get("BASS_COSINE_MODE", "fp32r")

_CACHED = {}

_MODES = {
    "fp32r": (mybir.dt.float32r, np.float32),
    "fp32": (mybir.dt.float32, np.float32),
    "fp16": (mybir.dt.float16, np.float16),
    "bf16": (mybir.dt.bfloat16, None),  # np dtype resolved via ml_dtypes
}


def _np_dtype(mode):
    if mode == "bf16":
        import ml_dtypes

        return ml_dtypes.bfloat16
    return _MODES[mode][1]


def _build_bass(mode):
    """Build + compile the single-core program (same NEFF runs on all 8 cores)."""
    nc = bacc.Bacc("TRN2", target_bir_lowering=False, debug=False,
                   num_devices=N_CORES)
    mmdt = _MODES[mode][0]
    fT_d = nc.dram_tensor("fT", [K, B], mmdt, kind="ExternalInput").ap()
    wT_d = nc.dram_tensor("wT", [K, N_SHARD], mmdt, kind="ExternalInput").ap()
    out_d = nc.dram_tensor("out", [B, N_SHARD], mybir.dt.float32,
                           kind="ExternalOutput").ap()

    wT_r = wT_d.rearrange("(c p) n -> p c n", p=128)   # [128, KC, N_SHARD]
    fT_r = fT_d.rearrange("(c p) b -> p c b", p=128)   # [128, KC, B]

    with tile.TileContext(nc) as tc:
        with (
            tc.tile_pool(name="fpool", bufs=1) as fpool,
            tc.tile_pool(name="wpool", bufs=3) as wpool,
            tc.tile_pool(name="opool", bufs=3) as opool,
            tc.tile_pool(name="psum", bufs=8, space="PSUM") as psum,
        ):
            fsb = fpool.tile([128, KC, B], mmdt)
            nc.sync.dma_start(fsb[:], fT_r[:])

            for n in range(N_CHUNKS):
                wsb = wpool.tile([128, KC, NT], mmdt)
                nc.sync.dma_start(wsb[:], wT_r[:, :, n * NT:(n + 1) * NT])

                for b in range(B // 128):
                    osb = opool.tile([128, NT], mybir.dt.float32)
                    for h in range(NT // 512):
                        ps = psum.tile([128, 512], mybir.dt.float32)
                        for c in range(KC):
                            nc.tensor.matmul(
                                ps[:],
                                fsb[:, c, b * 128:(b + 1) * 128],
                                wsb[:, c, h * 512:(h + 1) * 512],
                                start=(c == 0),
                                stop=(c == KC - 1),
                            )
                        nc.vector.tensor_copy(osb[:, h * 512:(h + 1) * 512], ps[:])
                    # output DMAs ride the ACT HWDGE ring so they don't
                    # queue behind the next chunk's input DMA on SP
                    nc.scalar.dma_start(
                        out_d[b * 128:(b + 1) * 128, n * NT:(n + 1) * NT], osb[:]
                    )
    nc.compile()
    return nc


def _run_spmd(nc, in_maps):
    last_exc = None
    for _ in range(3):  # device occasionally needs one recovery execute
        try:
            return run_bass_kernel_spmd(nc, in_maps, core_ids=list(range(N_CORES)))
        except Exception as e:  # noqa: BLE001
            last_exc = e
    raise last_exc


def kernel(features, weight, threshold):
    features = np.asarray(features, dtype=np.float32)
    weight = np.asarray(weight, dtype=np.float32)
    npdt = _np_dtype(MODE)

    f_norm = np.linalg.norm(features, axis=1, keepdims=True)
    f_hat = features / np.maximum(f_norm, EPS)
    fT = np.ascontiguousarray(f_hat.T).astype(npdt)          # [768, 256]

    w_norm = np.linalg.norm(weight, axis=1, keepdims=True)
    w_inv = (1.0 / np.maximum(w_norm, EPS)).astype(np.float32)

    shards = []
    for i in range(N_CORES):
        n0 = i * N_SHARD
        n1 = min(n0 + N_SHARD, N_FULL)
        s = np.zeros((K, N_SHARD), dtype=npdt)
        s[:, : n1 - n0] = (weight[n0:n1].T * w_inv[n0:n1].T).astype(npdt)
        shards.append(s)

    key = ("nc", MODE)
    if key not in _CACHED:
        _CACHED[key] = _build_bass(MODE)
    nc = _CACHED[key]

    in_maps = [{"fT": fT, "wT": shards[i]} for i in range(N_CORES)]
    res = _run_spmd(nc, in_maps)
    _CACHED["last_result"] = res

    out = np.empty((B, N_FULL + 1), dtype=np.float32)
    for i in range(N_CORES):
        n0 = i * N_SHARD
        n1 = min(n0 + N_SHARD, N_FULL)
        out[:, n0:n1] = res.results[i]["out"][:, : n1 - n0]
    out[:, N_FULL] = np.float32(threshold)
    return out


# revision 7
# speedup vs baseline: 1.6719x; 1.0032x over previous
"""Trainium2 Bass kernel for nn_CosineLayer (retrieval_knn).

Computes out = concat(normalize(features) @ normalize(weight).T, threshold_col).

Strategy (tensor/vocab parallel on the 434k concept axis, per sharding hint):
  - Host: L2-normalize features and weight rows (cheap one-pass prep), fold
    normalization into the weight, transpose shards to [K, N_shard] so the
    contraction dim lands on SBUF partitions, pad N to 8*54272.
  - Device (x8 SPMD): pure streaming matmul sim_shard = f_hatT.T @ w_hatT_shard
    (float32r: TF32-like TensorE mode, 1 cycle/row), PSUM accumulation over
    K=768 (6 chunks of 128), DVE copy PSUM->SBUF, DMA out.
  - Host: concat shard outputs, trim padding, append threshold column.
"""

import os

import numpy as np

import concourse.mybir as mybir
import concourse.tile as tile
from concourse import bacc
from concourse.bass_utils import run_bass_kernel_spmd

N_CORES = 8
B = 256              # feature rows
K = 768              # embedding dim
KC = K // 128        # 6 k-chunks of 128 partitions
N_FULL = 434056      # concept rows
N_SHARD = 54272      # = 106*512; 8*54272 = 434176 (pad 120)
NT = int(os.environ.get("BASS_COSINE_NT", "1024"))   # n-columns per chunk
N_CHUNKS = N_SHARD // NT
OUT_BATCH = int(os.environ.get("BASS_COSINE_OUT_BATCH", "1"))  # chunks per out-DMA
EPS = 1e-8

# weight/feature compute dtype. fp16 halves HBM traffic vs fp32/fp32r and,
# with fp32 PSUM accumulation, measures 1.2e-4 scale-relative absmax vs the
# fp32 reference (fp32r measures 6.4e-5 at 1.67x the runtime; bf16 2.4e-3).
MODE = os.environ.get("BASS_COSINE_MODE", "fp16")

_CACHED = {}

_MODES = {
    "fp32r": (mybir.dt.float32r, np.float32),
    "fp32": (mybir.dt.float32, np.float32),
    "fp16": (mybir.dt.float16, np.float16),
    "bf16": (mybir.dt.bfloat16, None),  # np dtype resolved via ml_dtypes
}


def _np_dtype(mode):
    if mode == "bf16":
        import ml_dtypes

        return ml_dtypes.bfloat16
    return _MODES[mode][1]


def _build_bass(mode):
    """Build + compile the single-core program (same NEFF runs on all 8 cores)."""
    assert N_CHUNKS % OUT_BATCH == 0, "OUT_BATCH must divide N_CHUNKS"
    nc = bacc.Bacc("TRN2", target_bir_lowering=False, debug=False,
                   num_devices=N_CORES)
    mmdt = _MODES[mode][0]
    fT_d = nc.dram_tensor("fT", [K, B], mmdt, kind="ExternalInput").ap()
    wT_d = nc.dram_tensor("wT", [K, N_SHARD], mmdt, kind="ExternalInput").ap()
    out_d = nc.dram_tensor("out", [B, N_SHARD], mybir.dt.float32,
                           kind="ExternalOutput").ap()

    wT_r = wT_d.rearrange("(c p) n -> p c n", p=128)   # [128, KC, N_SHARD]
    fT_r = fT_d.rearrange("(c p) b -> p c b", p=128)   # [128, KC, B]

    with tile.TileContext(nc) as tc:
        with (
            tc.tile_pool(name="fpool", bufs=1) as fpool,
            tc.tile_pool(name="wpool", bufs=3) as wpool,
            tc.tile_pool(name="opool", bufs=3) as opool,
            tc.tile_pool(name="psum", bufs=8, space="PSUM") as psum,
        ):
            fsb = fpool.tile([128, KC, B], mmdt)
            nc.sync.dma_start(fsb[:], fT_r[:])

            for g in range(N_CHUNKS // OUT_BATCH):
                osb = [
                    opool.tile([128, OUT_BATCH * NT], mybir.dt.float32,
                               name=f"osb{b}", tag=f"osb{b}")
                    for b in range(B // 128)
                ]
                for j in range(OUT_BATCH):
                    n = g * OUT_BATCH + j
                    wsb = wpool.tile([128, KC, NT], mmdt)
                    nc.sync.dma_start(wsb[:], wT_r[:, :, n * NT:(n + 1) * NT])

                    for b in range(B // 128):
                        for h in range(NT // 512):
                            ps = psum.tile([128, 512], mybir.dt.float32)
                            for c in range(KC):
                                nc.tensor.matmul(
                                    ps[:],
                                    fsb[:, c, b * 128:(b + 1) * 128],
                                    wsb[:, c, h * 512:(h + 1) * 512],
                                    start=(c == 0),
                                    stop=(c == KC - 1),
                                )
                            nc.vector.tensor_copy(
                                osb[b][:, j * NT + h * 512: j * NT + (h + 1) * 512],
                                ps[:],
                            )
                # output DMAs ride the ACT HWDGE ring so they don't
                # queue behind the next chunk's input DMA on SP
                n0 = g * OUT_BATCH * NT
                for b in range(B // 128):
                    nc.scalar.dma_start(
                        out_d[b * 128:(b + 1) * 128, n0:n0 + OUT_BATCH * NT], osb[b][:]
                    )
    nc.compile()
    return nc


def _run_spmd(nc, in_maps):
    last_exc = None
    for _ in range(3):  # device occasionally needs one recovery execute
        try:
            return run_bass_kernel_spmd(nc, in_maps, core_ids=list(range(N_CORES)))
        except Exception as e:  # noqa: BLE001
            last_exc = e
    raise last_exc


def kernel(features, weight, threshold):
    features = np.asarray(features, dtype=np.float32)
    weight = np.asarray(weight, dtype=np.float32)
    npdt = _np_dtype(MODE)

    f_norm = np.linalg.norm(features, axis=1, keepdims=True)
    f_hat = features / np.maximum(f_norm, EPS)
    fT = np.ascontiguousarray(f_hat.T).astype(npdt)          # [768, 256]

    w_norm = np.linalg.norm(weight, axis=1, keepdims=True)
    w_inv = (1.0 / np.maximum(w_norm, EPS)).astype(np.float32)

    shards = []
    for i in range(N_CORES):
        n0 = i * N_SHARD
        n1 = min(n0 + N_SHARD, N_FULL)
        s = np.zeros((K, N_SHARD), dtype=npdt)
        s[:, : n1 - n0] = (weight[n0:n1].T * w_inv[n0:n1].T).astype(npdt)
        shards.append(s)

    key = ("nc", MODE)
    if key not in _CACHED:
        _CACHED[key] = _build_bass(MODE)
    nc = _CACHED[key]

    in_maps = [{"fT": fT, "wT": shards[i]} for i in range(N_CORES)]
    res = _run_spmd(nc, in_maps)
    _CACHED["last_result"] = res

    out = np.empty((B, N_FULL + 1), dtype=np.float32)
    for i in range(N_CORES):
        n0 = i * N_SHARD
        n1 = min(n0 + N_SHARD, N_FULL)
        out[:, n0:n1] = res.results[i]["out"][:, : n1 - n0]
    out[:, N_FULL] = np.float32(threshold)
    return out
